# revision 20
# baseline (speedup 1.0000x reference)
"""Multi-head self-attention (B=4, S=2048, D=1024, H=16, Hd=64) on 8 TRN2 cores.

Sharding: tensor-parallel over heads for QKV+attention (core c owns heads
2c, 2c+1), token-parallel for the output projection (core c owns tokens
[b*2048 + hf*1024 + c*128, +128) for each half hf), bridged by two small
AllToAlls per batch (256 KB/rank, fired at mid-batch and batch-end) so
communication always completes a full batch before its consumers run.

All-bf16 datapath: x and weights are converted to bf16 on the host and
DMA'd directly into SBUF (no on-device casts). Per (batch, 512-query
stripe):
  - S^T pair: both heads' score matmuls issued back-to-back with
    tile_position (0,0)/(64,0) -> concurrent on the PE array (each uses
    only 64 contraction rows), into one [128,1024] PSUM tile.
  - one exp on ACT over both heads' scores ([128,1024], scale=1/8 folded).
  - AV per head with ve = [ones(64) | V(64)] so the softmax denominator
    lands in PSUM rows 0:64 and the AV rows in 64:128 for BOTH heads:
    every DVE op and the single partition_broadcast stay base-aligned.
  - normalize on DVE into per-head A^T tiles (rows 64:128 used).
Out-proj: lhsT = full (row-permuted) Wo, rhs = AllToAll-gathered A'^T
chunks, woven into the next batch's attention stripes; batch 3's halves
are consumed separately so the tail only waits for the last 256 KB A2A.
"""
import numpy as np

B, S, D, H, HD = 4, 2048, 1024, 16, 64
N_CORES = 8
TOK = B * S            # 8192
HPC = H // N_CORES     # 2 heads per core
CW = HPC * HD          # 128 cols per core
QS = 512               # query stripe
NKT = S // 128         # 16 kt chunks per batch
NQS = S // QS          # 4 q stripes per batch
TPC = S // N_CORES     # 256 tokens per (core, batch)
HP = TPC // 2          # 128 tokens per (core, batch, half)

_CACHE = {}


def _build():
    import concourse.bacc as bacc
    import concourse.mybir as mybir
    import concourse.tile as tile

    F32 = mybir.dt.float32
    F32R = mybir.dt.float32r
    BF16 = mybir.dt.bfloat16
    AF = mybir.ActivationFunctionType

    nc = bacc.Bacc(trn_type="TRN2", target_bir_lowering=False, debug=False,
                   num_devices=N_CORES)

    xT = nc.dram_tensor("xT", [D, TOK], BF16, kind="ExternalInput")
    wq = nc.dram_tensor("wq", [128, D], BF16, kind="ExternalInput")
    wk = nc.dram_tensor("wk", [128, D], BF16, kind="ExternalInput")
    wv = nc.dram_tensor("wv", [128, D], BF16, kind="ExternalInput")
    wo = nc.dram_tensor("wo", [128, 8 * D], BF16, kind="ExternalInput")
    bq = nc.dram_tensor("bq", [CW, 1], F32, kind="ExternalInput")
    bk = nc.dram_tensor("bk", [CW, 1], F32, kind="ExternalInput")
    bv = nc.dram_tensor("bv", [CW, 1], F32, kind="ExternalInput")
    bo = nc.dram_tensor("bo", [128, 8], F32, kind="ExternalInput")
    ident = nc.dram_tensor("ident", [128, 128], F32R, kind="ExternalInput")
    outT = nc.dram_tensor("outT", [D, B * TPC], F32, kind="ExternalOutput")

    with tile.TileContext(nc) as tc:
        with tc.tile_pool(name="sb", bufs=1) as sb, \
             tc.tile_pool(name="dram", bufs=1, space="DRAM") as dram:
            # ---------------- prologue: small weights, biases, constants --
            wq_s = sb.tile([128, D], BF16, tag="wq_s", name="wq_s")
            nc.sync.dma_start(wq_s[:], wq[:])
            wk_s = sb.tile([128, D], BF16, tag="wk_s", name="wk_s")
            nc.sync.dma_start(wk_s[:], wk[:])
            wv_s = sb.tile([128, D], BF16, tag="wv_s", name="wv_s")
            nc.sync.dma_start(wv_s[:], wv[:])
            identr = sb.tile([128, 128], F32R, tag="identr", name="identr")
            nc.sync.dma_start(identr[:], ident[:])

            bias_t = {}
            for bname, bdram, bshape in (("bq", bq, [CW, 1]),
                                         ("bk", bk, [CW, 1]),
                                         ("bv", bv, [CW, 1]),
                                         ("bo", bo, [128, 8])):
                bt_ = sb.tile(bshape, F32, tag=f"{bname}_t", name=f"{bname}_t")
                nc.sync.dma_start(bt_[:], bdram[:])
                bias_t[bname] = bt_

            a2a_in = {}
            a2a_out = {}
            for b in range(B):
                for hf in range(2):
                    a2a_in[(b, hf)] = dram.tile(
                        [N_CORES * 128, HP], BF16,
                        tag=f"a2ai{b}{hf}", name=f"a2ai{b}{hf}")
                    a2a_out[(b, hf)] = dram.tile(
                        [N_CORES * 128, HP], BF16,
                        tag=f"a2ao{b}{hf}", name=f"a2ao{b}{hf}")

            with tc.tile_pool(name="ps12", bufs=1, space="PSUM") as ps:
                qkv = {}
                xr_tiles = {}
                vext = {}
                at_tiles = {}
                ast_tiles = {}

                def emit_p1_loads(b, tb):
                    if tb == 0:
                        qkv[b] = (
                            sb.tile([128, S], BF16, tag="qt_sb", bufs=2,
                                    name=f"qt{b}"),
                            sb.tile([128, S], BF16, tag="kt_sb", bufs=2,
                                    name=f"kt{b}"),
                            sb.tile([128, S], F32R, tag="vt_sb", bufs=2,
                                    name=f"vt{b}"),
                        )
                        at_tiles[b] = (
                            sb.tile([128, S], BF16, tag="at0",
                                    bufs=2, name=f"at0_{b}"),
                            sb.tile([128, S], BF16, tag="at1",
                                    bufs=2, name=f"at1_{b}"),
                        )
                    g0 = b * S + tb * 512
                    xr = []
                    for k in range(8):
                        xk = sb.tile([128, 512], BF16, tag="xr", bufs=34,
                                     name=f"xr{b}_{tb}_{k}")
                        nc.sync.dma_start(
                            xk[:], xT[k * 128:(k + 1) * 128, g0:g0 + 512])
                        xr.append(xk)
                    xr_tiles[(b, tb)] = xr

                def emit_p1_group(b, tb, which):
                    qt, kt, vt = qkv[b]
                    xr = xr_tiles[(b, tb)]
                    w_, out_sb, bias = (
                        (wq_s, qt, bias_t["bq"]),
                        (wk_s, kt, bias_t["bk"]),
                        (wv_s, vt, bias_t["bv"]))[which]
                    pp = ps.tile([128, 512], F32, tag="proj", bufs=2,
                                 name=f"pp{b}_{tb}_{which}")
                    for k in range(8):
                        nc.tensor.matmul(
                            pp[:], w_[:, k * 128:(k + 1) * 128],
                            xr[k][:], start=(k == 0), stop=(k == 7))
                    nc.vector.tensor_scalar_add(
                        out_sb[:, tb * 512:(tb + 1) * 512], pp[:], bias[:])

                def emit_vext_chunk(b, tbi):
                    vt = qkv[b][2]
                    for ktc in range(4 * tbi, 4 * tbi + 4):
                        tp = ps.tile([128, 128], F32R, tag="proj", bufs=2,
                                     name=f"tp{b}_{ktc}")
                        nc.tensor.transpose(
                            tp[:], vt[:, ktc * 128:(ktc + 1) * 128],
                            identr[:])
                        # both heads: [ones | V] -> denom rows 0:64,
                        # AV rows 64:128 (everything stays base-aligned)
                        ve = sb.tile([128, 128], BF16, tag="vext",
                                     bufs=36, name=f"ve{b}_{ktc}")
                        nc.gpsimd.memset(ve[:, 0:64], 1.0)
                        nc.vector.tensor_copy(ve[:, 64:128], tp[:, 0:64])
                        ve2 = sb.tile([128, 128], BF16, tag="vext",
                                      bufs=36, name=f"v2{b}_{ktc}")
                        nc.gpsimd.memset(ve2[:, 0:64], 1.0)
                        nc.vector.tensor_copy(ve2[:, 64:128], tp[:, 64:128])
                        vext[(b, ktc, 0)] = ve
                        vext[(b, ktc, 1)] = ve2

                def emit_p2_stripe(b, qs_i, jobs):
                    qt, kt, vt = qkv[b]
                    at0, at1 = at_tiles[b]
                    q0 = qs_i * QS
                    pav = ps.tile([128, 1024], F32, tag="av", bufs=1,
                                  name=f"pav{b}_{qs_i}")

                    def emit_av(ktc, pt):
                        nc.tensor.matmul(
                            pav[:, 0:512], vext[(b, ktc, 0)][:],
                            pt[:, 0:512],
                            start=(ktc == 0), stop=(ktc == NKT - 1))
                        nc.tensor.matmul(
                            pav[:, 512:1024], vext[(b, ktc, 1)][:],
                            pt[:, 512:1024],
                            start=(ktc == 0), stop=(ktc == NKT - 1))

                    pts = []
                    for ktc in range(NKT):
                        for job in jobs.get(ktc, ()):
                            job()
                        s_ps = ps.tile([128, 1024], F32, tag="s", bufs=2,
                                       name=f"s{b}_{qs_i}_{ktc}")
                        nc.tensor.matmul(
                            s_ps[:, 0:512],
                            kt[0:64, ktc * 128:(ktc + 1) * 128],
                            qt[0:64, q0:q0 + 512],
                            start=True, stop=True, tile_position=(0, 0))
                        nc.tensor.matmul(
                            s_ps[:, 512:1024],
                            kt[64:128, ktc * 128:(ktc + 1) * 128],
                            qt[64:128, q0:q0 + 512],
                            start=True, stop=True, tile_position=(64, 0))
                        pt = sb.tile([128, 1024], BF16, tag="p_sb",
                                     bufs=3, name=f"p{b}_{qs_i}_{ktc}")
                        nc.scalar.activation(pt[:], s_ps[:], AF.Exp,
                                             scale=0.125)
                        pts.append(pt)
                        # AV lags 2 chunks behind so its exp is already
                        # done when the PE FIFO reaches it
                        if ktc >= 2:
                            emit_av(ktc - 2, pts[ktc - 2])
                    emit_av(NKT - 2, pts[NKT - 2])
                    emit_av(NKT - 1, pts[NKT - 1])
                    # pav rows 0:64 = denominators, rows 64:128 = AV
                    # (cols 0:512 = h0, cols 512:1024 = h1)
                    araw = sb.tile([128, 1024], F32, tag="araw", bufs=2,
                                   name=f"ar{b}_{qs_i}")
                    nc.vector.tensor_copy(araw[:], pav[:])
                    rcf = sb.tile([128, 1024], F32, tag="rcf", bufs=2,
                                  name=f"rcf{b}_{qs_i}")
                    nc.vector.reciprocal_approx_fast(rcf[:], araw[:])
                    bcs = sb.tile([128, 1024], F32, tag="bcs", bufs=2,
                                  name=f"bcs{b}_{qs_i}")
                    nc.gpsimd.partition_broadcast(bcs[:], rcf[0:1, :])
                    nc.vector.tensor_mul(at0[64:128, q0:q0 + QS],
                                         araw[64:128, 0:512],
                                         bcs[64:128, 0:512])
                    nc.vector.tensor_mul(at1[64:128, q0:q0 + QS],
                                         araw[64:128, 512:1024],
                                         bcs[64:128, 512:1024])
                    # rows r<64 of a shard = h1 (head 2c+1), r>=64 = h0
                    hf = qs_i // 2
                    for jj in range(4):
                        j = (qs_i % 2) * 4 + jj
                        tok = q0 + jj * HP
                        nc.sync.dma_start(
                            a2a_in[(b, hf)][j * 128:j * 128 + 64, :],
                            at1[64:128, tok:tok + HP])
                        nc.sync.dma_start(
                            a2a_in[(b, hf)][j * 128 + 64:(j + 1) * 128, :],
                            at0[64:128, tok:tok + HP])
                    if qs_i % 2 == 1:
                        nc.gpsimd.collective_compute(
                            "AllToAll", mybir.AluOpType.bypass,
                            replica_groups=[list(range(N_CORES))],
                            ins=[a2a_in[(b, hf)][:]],
                            outs=[a2a_out[(b, hf)][:]],
                        )

                def emit_ast_loads(b, parts):
                    ast = ast_tiles.get(b)
                    if ast is None:
                        ast = [sb.tile([128, TPC], BF16, tag="ast", bufs=18,
                                       name=f"ast{b}_{k}") for k in range(8)]
                        ast_tiles[b] = ast
                    for hf in parts:
                        for k in range(8):
                            nc.sync.dma_start(
                                ast[k][:, hf * HP:(hf + 1) * HP],
                                a2a_out[(b, hf)][k * 128:(k + 1) * 128, :])

                def emit_p3_ogroup(b, o, hf=None):
                    ast = ast_tiles[b]
                    c0, nc_ = (0, TPC) if hf is None else (hf * HP, HP)
                    po = ps.tile([128, TPC], F32, tag="proj", bufs=2,
                                 name=f"po{b}_{o}_{hf}")
                    for k in range(8):
                        nc.tensor.matmul(
                            po[:, 0:nc_],
                            wo_s[:, k * D + o * 128:k * D + (o + 1) * 128],
                            ast[k][:, c0:c0 + nc_],
                            start=(k == 0), stop=(k == 7))
                    ot = sb.tile([128, TPC], F32, tag="ot", bufs=3,
                                 name=f"ot{b}_{o}_{hf}")
                    nc.vector.tensor_scalar_add(ot[:, 0:nc_], po[:, 0:nc_],
                                                bias_t["bo"][:, o:o + 1])
                    nc.sync.dma_start(
                        outT[o * 128:(o + 1) * 128,
                             b * TPC + c0:b * TPC + c0 + nc_],
                        ot[:, 0:nc_])

                # ---------------- batch 0 fast-start ----------------------
                emit_p1_loads(0, 0)
                emit_p1_loads(0, 1)
                for w in (1, 2, 0):
                    emit_p1_group(0, 0, w)
                emit_vext_chunk(0, 0)
                # big Wo DMA deferred so batch-0 x loads win the queues
                wo_s = sb.tile([128, 8 * D], BF16, tag="wo_s", name="wo_s")
                for q in range(4):
                    nc.sync.dma_start(wo_s[:, q * 2048:(q + 1) * 2048],
                                      wo[:, q * 2048:(q + 1) * 2048])

                def sched(b):
                    """jobs[qs][ktc] for stripes of batch b."""
                    jobs = {qs: {} for qs in range(NQS)}

                    def add(qs, ktc, fn):
                        jobs[qs].setdefault(ktc, []).append(fn)

                    nb = b + 1
                    if b == 0:
                        # finish batch 0's own projections inside stripes 0/1
                        add(0, 0, lambda: emit_p1_group(0, 1, 1))
                        add(0, 2, lambda: emit_p1_group(0, 1, 2))
                        add(0, 3, lambda: emit_vext_chunk(0, 1))
                        add(0, 4, lambda: emit_p1_loads(0, 2))
                        add(0, 6, lambda: emit_p1_group(0, 2, 1))
                        add(0, 7, lambda: emit_p1_group(0, 2, 2))
                        add(0, 8, lambda: emit_vext_chunk(0, 2))
                        add(0, 9, lambda: emit_p1_loads(0, 3))
                        add(0, 10, lambda: emit_p1_group(0, 3, 1))
                        add(0, 11, lambda: emit_p1_group(0, 3, 2))
                        add(0, 12, lambda: emit_vext_chunk(0, 3))
                        add(0, 14, lambda: emit_p1_group(0, 1, 0))
                        add(1, 1, lambda: emit_p1_group(0, 2, 0))
                        add(1, 3, lambda: emit_p1_group(0, 3, 0))
                        add(1, 6, lambda: emit_p1_loads(1, 0))
                        add(1, 9, lambda: emit_p1_loads(1, 1))
                        add(1, 12, lambda: emit_p1_group(1, 0, 0))
                        add(1, 15, lambda: emit_p1_group(1, 0, 1))
                        add(2, 1, lambda: emit_p1_group(1, 0, 2))
                        add(2, 4, lambda: emit_p1_group(1, 1, 0))
                        add(2, 7, lambda: emit_p1_group(1, 1, 1))
                        add(2, 10, lambda: emit_p1_group(1, 1, 2))
                        add(2, 13, lambda: emit_p1_loads(1, 2))
                        add(2, 15, lambda: emit_vext_chunk(1, 0))
                        add(3, 1, lambda: emit_p1_group(1, 2, 0))
                        add(3, 3, lambda: emit_p1_group(1, 2, 1))
                        add(3, 5, lambda: emit_p1_group(1, 2, 2))
                        add(3, 7, lambda: emit_p1_loads(1, 3))
                        add(3, 9, lambda: emit_p1_group(1, 3, 0))
                        add(3, 11, lambda: emit_p1_group(1, 3, 1))
                        add(3, 13, lambda: emit_p1_group(1, 3, 2))
                        add(3, 15, lambda: emit_vext_chunk(1, 1))
                        return jobs

                    # b >= 1: weave next batch's projections + prev batch's
                    # out-proj (both halves landed a full batch ago).
                    add(0, 0, lambda pb=b - 1: emit_ast_loads(pb, (0, 1)))
                    for i in range(4):
                        add(1, 1 + 4 * i,
                            lambda pb=b - 1, o=i: emit_p3_ogroup(pb, o))
                        add(2, 1 + 4 * i,
                            lambda pb=b - 1, o=4 + i: emit_p3_ogroup(pb, o))
                    if nb < B:
                        add(0, 1, lambda: emit_p1_loads(nb, 0))
                        add(0, 4, lambda: emit_p1_loads(nb, 1))
                        add(0, 7, lambda: emit_p1_group(nb, 0, 0))
                        add(0, 10, lambda: emit_p1_group(nb, 0, 1))
                        add(0, 13, lambda: emit_p1_group(nb, 0, 2))
                        add(1, 3, lambda: emit_p1_group(nb, 1, 0))
                        add(1, 7, lambda: emit_p1_group(nb, 1, 1))
                        add(1, 11, lambda: emit_p1_group(nb, 1, 2))
                        add(1, 14, lambda: emit_p1_loads(nb, 2))
                        add(2, 3, lambda: emit_p1_group(nb, 2, 0))
                        add(2, 7, lambda: emit_p1_group(nb, 2, 1))
                        add(2, 11, lambda: emit_p1_group(nb, 2, 2))
                        add(2, 14, lambda: emit_p1_loads(nb, 3))
                        add(2, 15, lambda: emit_vext_chunk(nb, 0))
                        add(3, 1, lambda: emit_p1_group(nb, 3, 0))
                        add(3, 3, lambda: emit_p1_group(nb, 3, 1))
                        add(3, 5, lambda: emit_p1_group(nb, 3, 2))
                        add(3, 8, lambda: emit_vext_chunk(nb, 1))
                        add(3, 11, lambda: emit_vext_chunk(nb, 2))
                        add(3, 14, lambda: emit_vext_chunk(nb, 3))
                    else:
                        # batch 3 half-0: A2A fired at end of qs1, consume
                        # inside qs3
                        add(3, 4, lambda: emit_ast_loads(3, (0,)))
                        for i in range(8):
                            add(3, 6 + i, lambda o=i: emit_p3_ogroup(
                                3, o, hf=0))
                    return jobs

                for b in range(B):
                    jobs = sched(b)
                    for qs_i in range(NQS):
                        emit_p2_stripe(b, qs_i, jobs[qs_i])
                    if b == 0:
                        # b0's fast-start schedule has no qs3 room for these
                        emit_vext_chunk(1, 2)
                        emit_vext_chunk(1, 3)

                # ------------- tail: keep PE warm through the last A2A ----
                emit_ast_loads(3, (1,))
                warm = ps.tile([128, 512], F32, tag="proj", bufs=2,
                               name="warm")
                for i in range(70):
                    nc.tensor.matmul(warm[:], wq_s[:, 0:128],
                                     wq_s[:, 0:512], start=True, stop=True)
                for o in range(8):
                    emit_p3_ogroup(3, o, hf=1)

    nc.compile()
    return nc


def _get_nc():
    if "nc" not in _CACHE:
        _CACHE["nc"] = _build()
    return _CACHE["nc"]


def _make_in_maps(x, Wq, bq, Wk, bk, Wv, bv, Wo, bo):
    import ml_dtypes
    bf16 = ml_dtypes.bfloat16

    x = np.asarray(x, dtype=np.float32)
    Wq, Wk, Wv, Wo = (np.asarray(w, dtype=np.float32) for w in (Wq, Wk, Wv, Wo))
    bq, bk, bv, bo = (np.asarray(v, dtype=np.float32) for v in (bq, bk, bv, bo))

    xT = np.ascontiguousarray(x.reshape(TOK, D).T.astype(bf16))

    def warr(W, cs):
        # [128, 8*128]: chunk k cols <- W[k*128:(k+1)*128, cs]
        return np.ascontiguousarray(
            W[:, cs].reshape(8, 128, CW).transpose(1, 0, 2).reshape(128, D)
            .astype(bf16))

    # A2A-gathered A'^T row 128*i + r: r < 64 -> head 2i+1, r >= 64 ->
    # head 2i, dim r % 64.
    perm = np.empty(D, dtype=np.int64)
    for i in range(8):
        for r in range(128):
            h = 2 * i + (1 if r < 64 else 0)
            perm[128 * i + r] = h * 64 + (r % 64)
    wo_p = Wo[perm]  # [1024, 1024]
    wo_host = np.ascontiguousarray(
        wo_p.reshape(8, 128, D).transpose(1, 0, 2).reshape(128, 8 * D)
        .astype(bf16))

    bo_host = np.ascontiguousarray(bo.reshape(8, 128).T)

    in_maps = []
    for c in range(N_CORES):
        cs = slice(c * CW, (c + 1) * CW)
        in_maps.append({
            "xT": xT,
            "wq": warr(Wq, cs),
            "wk": warr(Wk, cs),
            "wv": warr(Wv, cs),
            "wo": wo_host,
            "bq": np.ascontiguousarray(bq[cs].reshape(CW, 1)),
            "bk": np.ascontiguousarray(bk[cs].reshape(CW, 1)),
            "bv": np.ascontiguousarray(bv[cs].reshape(CW, 1)),
            "bo": bo_host,
            "ident": np.eye(128, dtype=np.float32),
        })
    return in_maps


def kernel(x, Wq, bq, Wk, bk, Wv, bv, Wo, bo):
    from concourse import bass_utils

    in_maps = _make_in_maps(x, Wq, bq, Wk, bk, Wv, bv, Wo, bo)
    nc = _get_nc()
    res = bass_utils.run_bass_kernel_spmd(nc, in_maps,
                                          core_ids=list(range(N_CORES)))
    _CACHE["last_results"] = res

    out = np.empty((B, S, D), dtype=np.float32)
    for c in range(N_CORES):
        r = res.results[c]["outT"]  # [1024 odim, 4*256 token slots]
        for b in range(B):
            for hf in range(2):
                t0 = b * S + hf * (S // 2) + c * HP
                out.reshape(TOK, D)[t0:t0 + HP, :] = \
                    r[:, b * TPC + hf * HP:b * TPC + (hf + 1) * HP].T
    return out
